# revision 3
# baseline (speedup 1.0000x reference)
"""Trainium2 Bass kernel for nn_CapsuleSubLayer (capsule routing).

Math (per head h):
  uh[b,d,j] = sum_s W[h,d,j,s] * x[h,b,s,d]            (contraction over s)
  num_routing iterations of softmax / weighted-sum / squash / logit update
  out[b,d,n,h] = v[h,b,d]  (broadcast over n)

Sharding: 2 heads per core over 8 cores. Per core there are 128 (head,d)
slices, processed as 64 pairs q=(dd=2q, dd=2q+1).

Layouts (host pre-packs everything so every DMA is a flat [128, F] copy):
  xt[p, q, c, dl*64+b] = x[hl, b, c*128+p, d]   (dd=2q+dl, hl=dd//64, d=dd%64)
  wt[p, q, c, dl, j]   = W[hl, d, j, c*128+p]

Matmul: stationary = xt[:, q, c, :] (128 cols -> FWL fast weight load),
moving = wt[:, q, c, :, :] (32 cols = 16 j for each half). Output [128, 32]
accumulated over c into PSUM; the diagonal halves [0:64, 0:16] (=d0) and
[64:128, 16:32] (=d1) are the real uh values, the off-diagonal halves are
unused garbage that costs no extra PE time (LDW-bound).

Routing runs once over the whole core's uh [128, 64, 16] (partition =
(dl,b), free = (q, j)): reductions over j are free-axis reduces; the mean
over b uses a block-diagonal ones matmul on the PE which also leaves the
result replicated across the right partitions for the next softmax.
"""

import os
import sys

import numpy as np

for _p in ("/opt/trn_rl_repo",):
    if _p not in sys.path:
        sys.path.insert(0, _p)

from contextlib import ExitStack

import concourse.bass as bass
import concourse.tile as tile
from concourse import bacc, mybir
from concourse.bass_utils import run_bass_kernel_spmd

F32 = mybir.dt.float32
BF16 = mybir.dt.bfloat16
FP16 = mybir.dt.float16

H, B, S, D, N = 16, 64, 1024, 64, 16
NCORES = 8
H_LOC = H // NCORES  # 2 heads per core
C = S // 128  # 8 contraction chunks
Q = 64  # d-pairs per core
G = 8  # pairs per x DMA batch
NG = Q // G  # 8 batches

_cache = {}


def _build(num_routing: int):
    nc = bacc.Bacc(
        "TRN2", target_bir_lowering=False, debug=False, num_devices=NCORES
    )
    xt = nc.dram_tensor("xt", [128, Q, C, 128], FP16, kind="ExternalInput").ap()
    wt = nc.dram_tensor("wt", [128, Q, C, 2, N], FP16, kind="ExternalInput").ap()
    ones2 = nc.dram_tensor("ones2", [128, 128], FP16, kind="ExternalInput").ap()
    vout = nc.dram_tensor("vout", [128, Q], F32, kind="ExternalOutput").ap()

    # MM/DMA group sizes (pairs): small first so the PE starts early
    GROUPS = [2, 2, 4, 8, 8, 8, 8, 8, 8, 8]
    assert sum(GROUPS) == Q
    # routing chunk boundaries (pairs)
    RCHUNKS = [16, 16, 16, 16]
    assert sum(RCHUNKS) == Q

    with ExitStack() as ctx:
        tc = ctx.enter_context(tile.TileContext(nc))
        xpool = ctx.enter_context(tc.tile_pool(name="xp", bufs=4))
        wpool = ctx.enter_context(tc.tile_pool(name="wp", bufs=4))
        pspool = ctx.enter_context(tc.tile_pool(name="ps", bufs=1, space="PSUM"))
        blpool = ctx.enter_context(tc.tile_pool(name="bp", bufs=2, space="PSUM"))
        rpool = ctx.enter_context(tc.tile_pool(name="rt", bufs=2))
        spool = ctx.enter_context(tc.tile_pool(name="sm", bufs=4))
        singles = ctx.enter_context(tc.tile_pool(name="sg", bufs=1))

        ones_sb = singles.tile([128, 128], FP16)
        nc.sync.dma_start(out=ones_sb, in_=ones2)
        uh = singles.tile([128, Q, N], BF16)

        def routing(q0, RH, uhc, vout_slice):
            """num_routing iterations on uh[:, q0:q0+RH] ([128, RH, N])."""
            if num_routing > 1:
                bl_ps = blpool.tile([128, RH, N], F32, tag="bl")
            for it in range(num_routing):
                if it == 0:
                    s_raw = spool.tile([128, RH, 1], F32, tag="sr")
                    nc.vector.reduce_sum(s_raw, uhc, mybir.AxisListType.X)
                    scale = 1.0 / N
                else:
                    e = rpool.tile([128, RH, N], BF16, tag="e")
                    nc.scalar.activation(e, bl_ps, mybir.ActivationFunctionType.Exp)
                    esum = spool.tile([128, RH, 1], F32, tag="es")
                    nc.vector.reduce_sum(esum, e, mybir.AxisListType.X)
                    erec = spool.tile([128, RH, 1], F32, tag="er")
                    nc.vector.reciprocal(erec, esum)
                    cu = rpool.tile([128, RH, N], BF16, tag="cu")
                    nc.vector.tensor_mul(cu, e, uhc)
                    csum = spool.tile([128, RH, 1], F32, tag="cs")
                    nc.vector.reduce_sum(csum, cu, mybir.AxisListType.X)
                    s_raw = spool.tile([128, RH, 1], F32, tag="sr")
                    nc.vector.tensor_mul(s_raw, csum, erec)
                    scale = 1.0

                # squash: v = s*|s| / (1 + s^2), s = s_raw*scale
                m = spool.tile([128, RH, 1], F32, tag="m")
                nc.scalar.activation(
                    m, s_raw, mybir.ActivationFunctionType.Abs, scale=scale
                )
                msq = spool.tile([128, RH, 1], F32, tag="mq")
                nc.scalar.activation(
                    msq, s_raw, mybir.ActivationFunctionType.Square, scale=scale
                )
                if scale != 1.0:
                    s_sc = spool.tile([128, RH, 1], F32, tag="ssc")
                    nc.scalar.mul(s_sc, s_raw, scale)
                else:
                    s_sc = s_raw
                den = spool.tile([128, RH, 1], F32, tag="dn")
                nc.vector.tensor_scalar_add(den, msq, 1.0)
                rec = spool.tile([128, RH, 1], F32, tag="rc")
                nc.vector.reciprocal(rec, den)
                # t1 = m*s_sc runs in parallel with den/rec, shortening the
                # serial chain to msq -> den -> rec -> v
                t1 = spool.tile([128, RH, 1], F32, tag="t1")
                nc.vector.tensor_mul(t1, m, s_sc)
                v = spool.tile([128, RH, 1], F32, tag="v")
                nc.vector.tensor_mul(v, t1, rec)

                if it < num_routing - 1:
                    uv = rpool.tile([128, RH, N], FP16, tag="uv")
                    nc.vector.tensor_mul(uv, uhc, v.to_broadcast((128, RH, N)))
                    # ones_sb is block-diag (N/B) over the two 64-partition
                    # halves: accumulates bl += (N/B)*sum_b uh*v per half
                    nc.tensor.matmul(
                        bl_ps,
                        ones_sb,
                        uv,
                        start=(it == 0),
                        stop=(it == num_routing - 2),
                    )
                else:
                    vo = spool.tile([128, RH], F32, tag="vo")
                    nc.vector.tensor_copy(out=vo, in_=v[:, :, 0])
                    nc.sync.dma_start(out=vout_slice, in_=vo)

        uh_ps = pspool.tile([128, Q, 32], F32)
        # routing chunk starts, and the pair count at which each fires
        rq = []
        acc = 0
        for rc in RCHUNKS:
            rq.append((acc, rc, acc + rc))
            acc += rc
        next_chunk = 0

        q0 = 0
        for g, GS in enumerate(GROUPS):
            # just-in-time W chunk for this group's pairs, then the x chunk
            w_t = wpool.tile([128, GS, C, 2, N], FP16, tag="w", padded_shape=[128, 8, C, 2, N])
            nc.sync.dma_start(out=w_t, in_=wt[:, q0 : q0 + GS])
            x_t = xpool.tile([128, GS, C, 128], FP16, tag="x", padded_shape=[128, 8, C, 128])
            nc.sync.dma_start(out=x_t, in_=xt[:, q0 : q0 + GS])
            for qi in range(GS):
                q = q0 + qi
                for c in range(C):
                    nc.tensor.matmul(
                        uh_ps[:, q, :],
                        x_t[:, qi, c, :],
                        w_t[:, qi, c, :, :],
                        start=(c == 0),
                        stop=(c == C - 1),
                    )
            q0 += GS
            # fire any routing chunk whose pairs are now all computed: its
            # DVE work overlaps the remaining groups' DMAs/matmuls
            while next_chunk < len(rq) and rq[next_chunk][2] <= q0:
                a, rc, b = rq[next_chunk]
                nc.vector.tensor_copy(out=uh[0:64, a:b], in_=uh_ps[0:64, a:b, 0:N])
                nc.vector.tensor_copy(
                    out=uh[64:128, a:b], in_=uh_ps[64:128, a:b, N : 2 * N]
                )
                routing(a, rc, uh[:, a:b], vout[:, a:b])
                next_chunk += 1
        assert next_chunk == len(rq)
    nc.finalize()
    return nc


def _prep_core(x, W, k):
    # xt[p, q, c, dl*64+b] = x[hl, b, c*128+p, d], dd=2q+dl=hl*64+d
    xs = x[H_LOC * k : H_LOC * (k + 1)]  # [2, B, S, D] f32
    xdd = (
        xs.reshape(2, B, C, 128, D).transpose(0, 4, 2, 3, 1).reshape(128, C, 128, B)
    )  # [dd, c, p, b]
    xt = np.ascontiguousarray(
        xdd.reshape(Q, 2, C, 128, B).transpose(3, 0, 2, 1, 4), dtype=np.float16
    ).reshape(128, Q, C, 128)
    # wt[p, q, c, dl, n] = W[hl, d, n, c*128+p]
    ws = W[H_LOC * k : H_LOC * (k + 1)]  # [2, D, N, S]
    wdd = (
        ws.reshape(2, D, N, C, 128).transpose(4, 0, 1, 3, 2).reshape(128, 128, C, N)
    )  # [p, dd, c, n]
    wt = np.ascontiguousarray(
        wdd.reshape(128, Q, 2, C, N).transpose(0, 1, 3, 2, 4), dtype=np.float16
    ).reshape(128, Q, C, 2, N)
    return xt, wt


def _assemble(vouts):
    # vouts[k] is [128, Q]: row p=dl*64+b, col q -> v[2k + dd//64, b, dd%64]
    v_full = np.empty((H, B, D), dtype=np.float32)
    for k in range(NCORES):
        r = np.asarray(vouts[k]).reshape(2, B, Q)  # [dl, b, q]
        vdd = r.transpose(2, 0, 1).reshape(128, B)  # [dd, b]
        v_full[H_LOC * k : H_LOC * (k + 1)] = vdd.reshape(2, D, B).transpose(0, 2, 1)
    out = np.broadcast_to(v_full.transpose(1, 2, 0)[:, :, None, :], (B, D, N, H))
    return np.ascontiguousarray(out)


def kernel(x, W, num_routing):
    x = np.asarray(x, dtype=np.float32)
    W = np.asarray(W, dtype=np.float32)
    nr = int(num_routing)
    if nr == 0:
        return np.zeros((B, D, N, H), dtype=np.float32)
    if nr not in _cache:
        _cache[nr] = _build(nr)
    nc = _cache[nr]

    ones2 = np.zeros((128, 128), dtype=np.float16)
    ones2[:64, :64] = float(N) / B
    ones2[64:, 64:] = float(N) / B
    in_maps = []
    for k in range(NCORES):
        xt, wt = _prep_core(x, W, k)
        in_maps.append({"xt": xt, "wt": wt, "ones2": ones2})

    kernel.last_in_maps = in_maps
    res = run_bass_kernel_spmd(
        nc,
        in_maps,
        core_ids=list(range(NCORES)),
        trace=bool(int(os.environ.get("KERNEL_TRACE", "0"))),
    )
    kernel.last_result = res
    return _assemble([res.results[k]["vout"] for k in range(NCORES)])


# revision 5
# speedup vs baseline: 1.1428x; 1.1428x over previous
"""Trainium2 Bass kernel for nn_CapsuleSubLayer (capsule routing), v2.

Math (per head h):
  uh[b,d,j] = sum_s W[h,d,j,s] * x[h,b,s,d]            (contraction over s)
  num_routing iterations of softmax / weighted-sum / squash / logit update
  out[b,d,n,h] = v[h,b,d]  (broadcast over n)

Sharding: 2 heads per core over 8 cores. Per core there are 128 (head,d)
slices, processed as 64 pairs q=(dd=2q, dd=2q+1).

Layouts (host pre-packs everything so every DMA is a flat [128, F] copy):
  xt[p, q, c, dl*64+b] = x[hl, b, c*128+p, d]   (dd=2q+dl, hl=dd//64, d=dd%64)
  wt[p, q, c, dl, j]   = W[hl, d, j, c*128+p]

Matmul: stationary = xt[:, q, c, :] (128 cols -> FWL fast weight load),
moving = wt[:, q, c, :, :] (32 cols = 16 j for each half). Output [128, 32]
accumulated over c into PSUM; the diagonal halves [0:64, 0:16] (=d0) and
[64:128, 16:32] (=d1) are the real uh values, the off-diagonal halves are
unused garbage that costs no extra PE time (LDW-bound).

Routing runs once over the whole core's uh [128, 64, 16] (partition =
(dl,b), free = (q, j)): reductions over j are free-axis reduces; the mean
over b uses a block-diagonal ones matmul on the PE which also leaves the
result replicated across the right partitions for the next softmax.
"""

import os
import sys

import numpy as np

for _p in ("/opt/trn_rl_repo",):
    if _p not in sys.path:
        sys.path.insert(0, _p)

from contextlib import ExitStack

import concourse.bass as bass
import concourse.tile as tile
from concourse import bacc, mybir
from concourse.bass_utils import run_bass_kernel_spmd

F32 = mybir.dt.float32
BF16 = mybir.dt.bfloat16
FP16 = mybir.dt.float16

H, B, S, D, N = 16, 64, 1024, 64, 16
NCORES = 8
H_LOC = H // NCORES  # 2 heads per core
C = S // 128  # 8 contraction chunks
Q = 64  # d-pairs per core
G = 8  # pairs per x DMA batch
NG = Q // G  # 8 batches

_cache = {}


def _build(num_routing: int):
    nc = bacc.Bacc(
        "TRN2", target_bir_lowering=False, debug=False, num_devices=NCORES
    )
    xt = nc.dram_tensor("xt", [128, Q, C, 128], FP16, kind="ExternalInput").ap()
    wt = nc.dram_tensor("wt", [128, Q, C, 2, N], FP16, kind="ExternalInput").ap()
    ones2 = nc.dram_tensor("ones2", [128, 128], FP16, kind="ExternalInput").ap()
    vout = nc.dram_tensor("vout", [128, Q], F32, kind="ExternalOutput").ap()

    # MM/DMA group sizes (pairs): small first so the PE starts early
    GROUPS = [2, 2, 4, 8, 8, 8, 8, 8, 8, 8]
    assert sum(GROUPS) == Q
    # routing chunk boundaries (pairs)
    RCHUNKS = [16, 16, 16, 8, 8]
    assert sum(RCHUNKS) == Q

    with ExitStack() as ctx:
        tc = ctx.enter_context(tile.TileContext(nc))
        xpool = ctx.enter_context(tc.tile_pool(name="xp", bufs=4))
        wpool = ctx.enter_context(tc.tile_pool(name="wp", bufs=4))
        pspool = ctx.enter_context(tc.tile_pool(name="ps", bufs=1, space="PSUM"))
        blpool = ctx.enter_context(tc.tile_pool(name="bp", bufs=2, space="PSUM"))
        rpool = ctx.enter_context(tc.tile_pool(name="rt", bufs=2))
        spool = ctx.enter_context(tc.tile_pool(name="sm", bufs=4))
        singles = ctx.enter_context(tc.tile_pool(name="sg", bufs=1))

        ones_sb = singles.tile([128, 128], FP16)
        nc.sync.dma_start(out=ones_sb, in_=ones2)
        uh = singles.tile([128, Q, N], BF16)

        def routing(q0, RH, uhc, vout_slice):
            """num_routing iterations on uh[:, q0:q0+RH] ([128, RH, N])."""
            if num_routing > 1:
                bl_ps = blpool.tile([128, RH, N], F32, tag="bl")
            for it in range(num_routing):
                # division-free squash: with p = sum_n c_raw*uh and
                # q = sum_n c_raw (c_raw = exp(bl), uniform at it 0),
                # squash(p/q) = p*|p| / (p^2 + q^2) exactly.
                if it == 0:
                    p = spool.tile([128, RH, 1], F32, tag="p")
                    nc.vector.reduce_sum(p, uhc, mybir.AxisListType.X)
                    qsq = float(N) * N
                else:
                    e = rpool.tile([128, RH, N], BF16, tag="e")
                    nc.scalar.activation(e, bl_ps, mybir.ActivationFunctionType.Exp)
                    esum = spool.tile([128, RH, 1], F32, tag="es")
                    nc.vector.reduce_sum(esum, e, mybir.AxisListType.X)
                    qsq = spool.tile([128, RH, 1], F32, tag="qq")
                    nc.vector.tensor_mul(qsq, esum, esum)
                    cu = rpool.tile([128, RH, N], BF16, tag="cu")
                    nc.vector.tensor_mul(cu, e, uhc)
                    p = spool.tile([128, RH, 1], F32, tag="p")
                    nc.vector.reduce_sum(p, cu, mybir.AxisListType.X)

                psq = spool.tile([128, RH, 1], F32, tag="pq")
                nc.vector.tensor_mul(psq, p, p)
                m = spool.tile([128, RH, 1], F32, tag="m")
                nc.vector.scalar_tensor_tensor(
                    m, p, -1.0, p, mybir.AluOpType.mult, mybir.AluOpType.max
                )
                den = spool.tile([128, RH, 1], F32, tag="dn")
                if it == 0:
                    nc.vector.tensor_scalar_add(den, psq, qsq)
                else:
                    nc.vector.tensor_add(den, psq, qsq)
                rec = spool.tile([128, RH, 1], F32, tag="rc")
                nc.vector.reciprocal(rec, den)
                # t1 = p*|p| runs in parallel with den/rec
                t1 = spool.tile([128, RH, 1], F32, tag="t1")
                nc.vector.tensor_mul(t1, p, m)
                v = spool.tile([128, RH, 1], F32, tag="v")
                nc.vector.tensor_mul(v, t1, rec)

                if it < num_routing - 1:
                    uv = rpool.tile([128, RH, N], FP16, tag="uv")
                    nc.vector.tensor_mul(uv, uhc, v.to_broadcast((128, RH, N)))
                    # ones_sb is block-diag (N/B) over the two 64-partition
                    # halves: accumulates bl += (N/B)*sum_b uh*v per half
                    # stop=True per update: a no-op on HW, but it closes the
                    # PSUM accumulation group so the next exp may read bl_ps
                    nc.tensor.matmul(
                        bl_ps,
                        ones_sb,
                        uv,
                        start=(it == 0),
                        stop=True,
                        skip_group_check=(it > 0),
                    )
                else:
                    vo = spool.tile([128, RH], F32, tag="vo")
                    nc.vector.tensor_copy(out=vo, in_=v[:, :, 0])
                    nc.sync.dma_start(out=vout_slice, in_=vo)

        uh_ps = pspool.tile([128, Q, 32], F32)
        # routing chunk starts, and the pair count at which each fires
        rq = []
        acc = 0
        for rc in RCHUNKS:
            rq.append((acc, rc, acc + rc))
            acc += rc
        next_chunk = 0

        q0 = 0
        for g, GS in enumerate(GROUPS):
            # just-in-time W chunk for this group's pairs, then the x chunk
            w_t = wpool.tile([128, GS, C, 2, N], FP16, tag="w", padded_shape=[128, 8, C, 2, N])
            nc.sync.dma_start(out=w_t, in_=wt[:, q0 : q0 + GS])
            x_t = xpool.tile([128, GS, C, 128], FP16, tag="x", padded_shape=[128, 8, C, 128])
            nc.sync.dma_start(out=x_t, in_=xt[:, q0 : q0 + GS])
            for qi in range(GS):
                q = q0 + qi
                for c in range(C):
                    nc.tensor.matmul(
                        uh_ps[:, q, :],
                        x_t[:, qi, c, :],
                        w_t[:, qi, c, :, :],
                        start=(c == 0),
                        stop=(c == C - 1),
                    )
            q0 += GS
            # fire any routing chunk whose pairs are now all computed: its
            # DVE work overlaps the remaining groups' DMAs/matmuls
            while next_chunk < len(rq) and rq[next_chunk][2] <= q0:
                a, rc, b = rq[next_chunk]
                nc.vector.tensor_copy(out=uh[0:64, a:b], in_=uh_ps[0:64, a:b, 0:N])
                nc.vector.tensor_copy(
                    out=uh[64:128, a:b], in_=uh_ps[64:128, a:b, N : 2 * N]
                )
                routing(a, rc, uh[:, a:b], vout[:, a:b])
                next_chunk += 1
        assert next_chunk == len(rq)
    nc.finalize()
    return nc


def _prep_core(x, W, k):
    # xt[p, q, c, dl*64+b] = x[hl, b, c*128+p, d], dd=2q+dl=hl*64+d
    xs = x[H_LOC * k : H_LOC * (k + 1)]  # [2, B, S, D] f32
    xdd = (
        xs.reshape(2, B, C, 128, D).transpose(0, 4, 2, 3, 1).reshape(128, C, 128, B)
    )  # [dd, c, p, b]
    xt = np.ascontiguousarray(
        xdd.reshape(Q, 2, C, 128, B).transpose(3, 0, 2, 1, 4), dtype=np.float16
    ).reshape(128, Q, C, 128)
    # wt[p, q, c, dl, n] = W[hl, d, n, c*128+p]
    ws = W[H_LOC * k : H_LOC * (k + 1)]  # [2, D, N, S]
    wdd = (
        ws.reshape(2, D, N, C, 128).transpose(4, 0, 1, 3, 2).reshape(128, 128, C, N)
    )  # [p, dd, c, n]
    wt = np.ascontiguousarray(
        wdd.reshape(128, Q, 2, C, N).transpose(0, 1, 3, 2, 4), dtype=np.float16
    ).reshape(128, Q, C, 2, N)
    return xt, wt


def _assemble(vouts):
    # vouts[k] is [128, Q]: row p=dl*64+b, col q -> v[2k + dd//64, b, dd%64]
    v_full = np.empty((H, B, D), dtype=np.float32)
    for k in range(NCORES):
        r = np.asarray(vouts[k]).reshape(2, B, Q)  # [dl, b, q]
        vdd = r.transpose(2, 0, 1).reshape(128, B)  # [dd, b]
        v_full[H_LOC * k : H_LOC * (k + 1)] = vdd.reshape(2, D, B).transpose(0, 2, 1)
    out = np.broadcast_to(v_full.transpose(1, 2, 0)[:, :, None, :], (B, D, N, H))
    return np.ascontiguousarray(out)


def kernel(x, W, num_routing):
    x = np.asarray(x, dtype=np.float32)
    W = np.asarray(W, dtype=np.float32)
    nr = int(num_routing)
    if nr == 0:
        return np.zeros((B, D, N, H), dtype=np.float32)
    if nr not in _cache:
        _cache[nr] = _build(nr)
    nc = _cache[nr]

    ones2 = np.zeros((128, 128), dtype=np.float16)
    ones2[:64, :64] = float(N) / B
    ones2[64:, 64:] = float(N) / B
    in_maps = []
    for k in range(NCORES):
        xt, wt = _prep_core(x, W, k)
        in_maps.append({"xt": xt, "wt": wt, "ones2": ones2})

    kernel.last_in_maps = in_maps
    res = run_bass_kernel_spmd(
        nc,
        in_maps,
        core_ids=list(range(NCORES)),
        trace=bool(int(os.environ.get("KERNEL_TRACE", "0"))),
    )
    kernel.last_result = res
    return _assemble([res.results[k]["vout"] for k in range(NCORES)])
